# revision 52
# baseline (speedup 1.0000x reference)
"""Trainium2 Bass kernel for the Canny-edge + 1x1-conv module.

Sharding: 8 cores = 4 images x 2 row-halves. Each core computes Canny on its
half (3 independent 128-row tiles with halos, K=5 hysteresis iterations) and
streams the fused concat+1x1conv+bias+relu output (32 MB/core) back to HBM.

All vertical (partition-axis) +-1 shifts go through TensorEngine matmuls with
constant banded matrices (compute-engine APs must start at partition 0).
Binary dilation = (3x3 box-sum >= 1), where the vertical part of the box-sum
is a tridiagonal matmul.

Self-contained: hardcodes all shapes; callable as kernel(x=..., Wc=..., b=...).
"""
import numpy as np

import concourse.bass as bass
import concourse.bacc as bacc
import concourse.mybir as mybir
import concourse.tile as tile
from concourse.bass_utils import run_bass_kernel_spmd

F32 = mybir.dt.float32
F16 = mybir.dt.float16
OP = mybir.AluOpType
ACT = mybir.ActivationFunctionType

B, C, H, W = 4, 3, 512, 512
WP = W + 2            # column-padded width
HS = 274              # shard rows: image rows [S-9, S+265)
K_HYST = 4
T_Q = [0, 112, 146]   # canny tile start rows within the shard
N_CHUNK = 32          # output chunks of 8 rows each
MAGIC = 8388608.0     # 2^23: f32 round-to-int trick
T1 = 0.4142135623730951   # tan(22.5 deg)
T2 = 2.414213562373095    # tan(67.5 deg)

LAST_RESULT = None    # BassKernelResults of the most recent run (for test.py)


def _chunk_map(k):
    """output chunk k (rows 8k..8k+8) -> (canny tile idx, partition start)"""
    if k <= 13:
        return 0, 8 * k + 9
    if k <= 27:
        return 1, 8 * k - 103
    return 2, 8 * k - 137


def _canny_gen(nc, pools, xs_param, mask_sb, mats, t, edge):
    """Generator emitting the Canny ops for shard rows [T_Q[t], T_Q[t]+128);
    yields between stages so the driver can interleave tiles / conv chunks.

    Ops are emitted per column-segment (L/R halves) so the serial chain
    pipelines across engines (DVE on one half while PE works the other).
    Padded-coordinate segments: [(1,258), (258,513)]; unpadded tiles use
    [a-1, b-1).
    """
    scr = pools["scratch"]
    cps = pools["cpsum"]
    xtLR = pools["xt"][t]          # [xtL [128,3*257], xtR [128,3*255]]
    SEG = [(1, 258), (258, 513)]

    # ---- gray = trunc(0.2989 x0 + 0.587 x1 + 0.114 x2)  (f32, exact) ----
    gray = scr.tile([128, W], F32, tag="gray")
    ground = scr.tile([128, W], F32, tag="ground")
    cmp = scr.tile([128, W], F16, tag="cmp")
    g = scr.tile([128, WP], F16, tag="g")
    for si, (a, b) in enumerate(SEG):
        u = slice(a - 1, b - 1)
        xts = xtLR[si]
        n = b - a
        nc.vector.tensor_scalar_mul(gray[:, u], xts[:, 0:n], 0.2989)
        nc.vector.scalar_tensor_tensor(gray[:, u], xts[:, n:2 * n], 0.587, gray[:, u], OP.mult, OP.add)
        nc.vector.scalar_tensor_tensor(gray[:, u], xts[:, 2 * n:3 * n], 0.114, gray[:, u], OP.mult, OP.add)
        nc.vector.tensor_scalar(ground[:, u], gray[:, u], MAGIC, MAGIC, OP.add, OP.subtract)
        nc.vector.tensor_tensor(cmp[:, u], ground[:, u], gray[:, u], OP.is_gt)
        nc.vector.tensor_tensor(g[:, a:b], ground[:, u], cmp[:, u], OP.subtract)
    nc.vector.tensor_copy(g[:, 0:1], g[:, 2:3])        # reflect cols
    nc.vector.tensor_copy(g[:, 513:514], g[:, 511:512])
    yield

    # ---- sobel: horizontal parts on DVE, vertical 3-taps via matmul ----
    dcol = scr.tile([128, W], F16, tag="dcol")
    hsm = scr.tile([128, W], F16, tag="hsm")
    gx = scr.tile([128, WP], F16, tag="gx")
    gy = scr.tile([128, WP], F16, tag="gy")
    ax = scr.tile([128, WP], F16, tag="ax")
    ay = scr.tile([128, WP], F16, tag="ay")
    pr = scr.tile([128, WP], F16, tag="pr")
    for (a, b) in SEG:
        u = slice(a - 1, b - 1)
        nc.vector.tensor_sub(dcol[:, u], g[:, a + 1:b + 1], g[:, a - 1:b - 1])
        nc.vector.scalar_tensor_tensor(hsm[:, u], g[:, a:b], 2.0, g[:, a - 1:b - 1], OP.mult, OP.add)
        nc.vector.tensor_add(hsm[:, u], hsm[:, u], g[:, a + 1:b + 1])
    yield
    for (a, b) in SEG:
        u = slice(a - 1, b - 1)
        n = b - a
        ps_gx = cps.tile([128, n], F32, tag="cps", padded_shape=[128, 257])
        nc.tensor.matmul(ps_gx[:, :], mats["tri121"][:, :], dcol[:, u], start=True, stop=True)
        ps_gy = cps.tile([128, n], F32, tag="cps", padded_shape=[128, 257])
        nc.tensor.matmul(ps_gy[:, :], mats["trim101"][:, :], hsm[:, u], start=True, stop=True)
        nc.vector.tensor_copy(gx[:, a:b], ps_gx[:, :])
        nc.scalar.activation(ax[:, a:b], ps_gx[:, :], ACT.Abs)
        nc.vector.tensor_copy(gy[:, a:b], ps_gy[:, :])
        nc.scalar.activation(ay[:, a:b], ps_gy[:, :], ACT.Abs)
        # scale one factor by 2^-6 (exact) so |pr| <= 64516 stays finite in fp16
        nc.vector.scalar_tensor_tensor(pr[:, a:b], gx[:, a:b], 0.015625, gy[:, a:b],
                                       OP.mult, OP.mult)
    yield

    # ---- mag (+ boundary mask) and shifted copies via shift matmuls ----
    mag = scr.tile([128, WP], F16, tag="mag")
    magu = scr.tile([128, WP], F16, tag="magu")
    magd = scr.tile([128, WP], F16, tag="magd")
    U8 = mybir.dt.uint8
    c0 = scr.tile([128, WP], U8, tag="c0")
    c2 = scr.tile([128, WP], U8, tag="c2")
    c45 = scr.tile([128, WP], U8, tag="c45")
    nc.vector.memset(mag[:, 0:1], 0.0)
    nc.vector.memset(mag[:, 513:514], 0.0)
    nc.vector.memset(magu[:, 0:1], 0.0)
    nc.vector.memset(magu[:, 513:514], 0.0)
    nc.vector.memset(magd[:, 0:1], 0.0)
    nc.vector.memset(magd[:, 513:514], 0.0)
    for (a, b) in SEG:
        nc.vector.tensor_add(mag[:, a:b], ax[:, a:b], ay[:, a:b])
        nc.vector.tensor_scalar(mag[:, a:b], mag[:, a:b], mask_sb[:, t:t + 1], None, OP.mult)
    yield
    for (a, b) in SEG:
        n = b - a
        ps_mu = cps.tile([128, n], F32, tag="cps", padded_shape=[128, 257])
        nc.tensor.matmul(ps_mu[:, :], mats["shup"][:, :], mag[:, a:b], start=True, stop=True)
        ps_md = cps.tile([128, n], F32, tag="cps", padded_shape=[128, 257])
        nc.tensor.matmul(ps_md[:, :], mats["shdn"][:, :], mag[:, a:b], start=True, stop=True)
        nc.vector.tensor_copy(magu[:, a:b], ps_mu[:, :])
        nc.vector.tensor_copy(magd[:, a:b], ps_md[:, :])
        nc.vector.scalar_tensor_tensor(c0[:, a:b], ax[:, a:b], T1, ay[:, a:b], OP.mult, OP.is_gt)
        nc.vector.scalar_tensor_tensor(c2[:, a:b], ax[:, a:b], T2, ay[:, a:b], OP.mult, OP.is_lt)
        nc.vector.tensor_scalar(c45[:, a:b], pr[:, a:b], 0.0, None, OP.is_gt)
    yield

    # ---- NMS neighbors via predicated copies (precedence: c45 < c2 < c0) ----
    n1 = scr.tile([128, WP], F16, tag="n1")
    n2 = scr.tile([128, WP], F16, tag="n2")
    q = scr.tile([128, WP], F16, tag="q")
    nms = scr.tile([128, WP], F16, tag="nms")
    strong = scr.tile([128, WP], F16, tag="strong")
    weak = scr.tile([128, WP], F16, tag="weak")
    for (a, b) in SEG:
        def sh(dy, dx):
            m = {-1: magd, 0: mag, 1: magu}[dy]
            return m[:, a + dx:b + dx]
        nc.vector.tensor_copy(n1[:, a:b], sh(-1, -1))
        nc.vector.tensor_copy(n2[:, a:b], sh(1, 1))
        for (c, a1, a2) in ((c45, (-1, 1), (1, -1)), (c2, (1, 0), (-1, 0)), (c0, (0, 1), (0, -1))):
            nc.vector.copy_predicated(n1[:, a:b], c[:, a:b], sh(*a1))
            nc.vector.copy_predicated(n2[:, a:b], c[:, a:b], sh(*a2))
        yield
        nc.vector.tensor_max(q[:, a:b], n1[:, a:b], n2[:, a:b])
        nc.vector.tensor_tensor(q[:, a:b], mag[:, a:b], q[:, a:b], OP.is_ge)
        nc.vector.tensor_mul(nms[:, a:b], mag[:, a:b], q[:, a:b])
        nc.vector.tensor_scalar(strong[:, a:b], nms[:, a:b], 150.0, None, OP.is_gt)
        nc.vector.tensor_scalar(weak[:, a:b], nms[:, a:b], 50.0, None, OP.is_gt)
    nc.vector.memset(strong[:, 0:1], 0.0)
    nc.vector.memset(strong[:, 513:514], 0.0)
    yield

    # ---- hysteresis: s' = weak AND (3x3 box-sum of s >= 1), K iterations.
    # Box-sum via 3 accumulating matmuls over column-shifted views (PE-only).
    sA = scr.tile([128, WP], F16, tag="sA")
    sB = scr.tile([128, WP], F16, tag="sB")
    for sbuf_t in (sA, sB):
        nc.vector.memset(sbuf_t[:, 0:1], 0.0)
        nc.vector.memset(sbuf_t[:, 513:514], 0.0)
    cur = strong
    for it in range(K_HYST):
        nxt = sA if (it % 2 == 0) else sB
        for (a, b) in SEG:
            n = b - a
            ps_h = cps.tile([128, n], F32, tag="cps", padded_shape=[128, 257])
            nc.tensor.matmul(ps_h[:, :], mats["tri111"][:, :], cur[:, a - 1:b - 1], start=True, stop=False)
            nc.tensor.matmul(ps_h[:, :], mats["tri111"][:, :], cur[:, a:b], start=False, stop=False)
            nc.tensor.matmul(ps_h[:, :], mats["tri111"][:, :], cur[:, a + 1:b + 1], start=False, stop=True)
            nc.vector.scalar_tensor_tensor(nxt[:, a:b], ps_h[:, :], 0.5, weak[:, a:b],
                                           OP.is_ge, OP.mult)
        cur = nxt
        yield

    for (a, b) in SEG:
        nc.vector.tensor_scalar(edge[:, a - 1:b - 1], cur[:, a:b], 255.0, None, OP.mult)


def build_nc():
    nc = bacc.Bacc("TRN2", target_bir_lowering=False)
    xs_param = nc.declare_dram_parameter("xs", [3, HS, W], F32, isOutput=False)
    xb_param = nc.declare_dram_parameter("xb", [16, 6, 4096], F16, isOutput=False)
    wt_param = nc.declare_dram_parameter("wt", [8, 128], F32, isOutput=False)
    bias_param = nc.declare_dram_parameter("bias", [128, 1], F32, isOutput=False)
    mask_param = nc.declare_dram_parameter("mask", [3, 128], F32, isOutput=False)
    mats_param = nc.declare_dram_parameter("mats", [128, 5 * 128], F16, isOutput=False)
    out_param = nc.declare_dram_parameter("out", [8, 128, 8192], F32, isOutput=True)

    MAT_NAMES = ["tri121", "trim101", "shup", "shdn", "tri111"]

    with tile.TileContext(nc) as tc:
        import contextlib
        with contextlib.ExitStack() as ctx:
            const = ctx.enter_context(tc.tile_pool(name="const", bufs=1))
            scratch = ctx.enter_context(tc.tile_pool(name="scratch", bufs=2))
            epool = ctx.enter_context(tc.tile_pool(name="edges", bufs=1))
            rhs_pool = ctx.enter_context(tc.tile_pool(name="rhs", bufs=2))
            stage_pool = ctx.enter_context(tc.tile_pool(name="stage", bufs=4))
            psum_pool = ctx.enter_context(tc.tile_pool(name="psum", bufs=2, space="PSUM"))
            cpsum_pool = ctx.enter_context(tc.tile_pool(name="cpsum", bufs=3, space="PSUM"))
            pools = {"scratch": scratch, "cpsum": cpsum_pool}

            wt_sb = const.tile([8, 128], F32, tag="wt")
            lhsT = const.tile([8, 128], F16, tag="lhsT")
            bias_sb = const.tile([128, 1], F32, tag="bias")
            mask_sb = const.tile([128, 3], F32, tag="mask")
            mats_sb = const.tile([128, 5 * 128], F16, tag="mats")
            # canny x tiles first in the sync FIFO (they gate the critical path);
            # split L/R so the first gray ops start as soon as the L half lands
            xts = []
            for t in range(3):
                pair = []
                for si, (ca, cb) in enumerate(((0, 257), (257, 512))):
                    n = cb - ca
                    xt = const.tile([128, 3 * n], F32, tag=f"xt{t}_{si}", name=f"xt{t}_{si}")
                    nc.sync.dma_start(
                        xt[:, :].rearrange("p (c w) -> p c w", c=3),
                        xs_param[:, T_Q[t]:T_Q[t] + 128, ca:cb].rearrange("c h w -> h c w"))
                    pair.append(xt)
                xts.append(pair)
            pools["xt"] = xts
            nc.scalar.dma_start(mats_sb[:, :], mats_param[:, :])
            nc.scalar.dma_start(wt_sb[:, :], wt_param[:, :])
            nc.scalar.dma_start(bias_sb[:, :], bias_param[:, :])
            nc.scalar.dma_start(mask_sb[:, :], mask_param.rearrange("t p -> p t"))
            nc.vector.tensor_copy(lhsT[:, :], wt_sb[:, :])
            mats = {nm: mats_sb[:, 128 * i:128 * (i + 1)] for i, nm in enumerate(MAT_NAMES)}

            edges = [epool.tile([128, W], F16, tag=f"edge{t}", name=f"edge{t}")
                     for t in range(3)]

            # relu fills handled by DVE per superchunk (DVE is canny-busy early)
            DVE_FILLS = {0: (), 1: (), 2: (), 3: (3,), 4: (2, 5), 5: (2, 5), 6: (1, 4, 7), 7: (0, 2, 4, 6)}

            def emit_superchunk(K):
                for half in range(2):
                    KH = 2 * K + half
                    rhs = rhs_pool.tile([8, 4096], F16, tag="rhs")
                    nc.sync.dma_start(rhs[0:6, :], xb_param[KH])
                    for ch in range(2):
                        k = 2 * KH + ch
                        t, p0 = _chunk_map(k)
                        for gg in range(2):
                            nc.sync.dma_start(
                                rhs[6 + gg:7 + gg, 2048 * ch:2048 * (ch + 1)]
                                .rearrange("one (h w) -> one h w", h=4),
                                edges[t][p0 + 4 * gg:p0 + 4 * gg + 4, :],
                            )
                    stage = stage_pool.tile([128, 4096], F32, tag="stage")
                    for jj4 in range(4):
                        psum = psum_pool.tile([128, 1024], F32, tag="psum")
                        for j in range(2):
                            nc.tensor.matmul(psum[:, 512 * j:512 * (j + 1)], lhsT[:, :],
                                             rhs[:, 1024 * jj4 + 512 * j:1024 * jj4 + 512 * (j + 1)],
                                             start=True, stop=True)
                        o0 = 1024 * jj4
                        if half * 4 + jj4 in DVE_FILLS[K]:
                            nc.vector.tensor_scalar(stage[:, o0:o0 + 1024], psum[:, :],
                                                    bias_sb[:, :], 0.0, OP.add, OP.max)
                        else:
                            nc.scalar.activation(stage[:, o0:o0 + 1024], psum[:, :],
                                                 ACT.Relu, bias=bias_sb[:, :])
                    eng = nc.scalar if KH % 2 == 0 else nc.gpsimd
                    eng.dma_start(out_param[K, :, 4096 * half:4096 * (half + 1)], stage[:, :])

            def drain(gen, n=10**9):
                for _ in range(n):
                    if next(gen, "done") == "done":
                        return True
                return False

            g0 = _canny_gen(nc, pools, xs_param, mask_sb, mats, 0, edges[0])
            g1 = _canny_gen(nc, pools, xs_param, mask_sb, mats, 1, edges[1])
            g2 = _canny_gen(nc, pools, xs_param, mask_sb, mats, 2, edges[2])
            drain(g0)
            # interleave canny T1 with superchunks 0-2 (T0-backed)
            for K in range(0, 3):
                drain(g1, 4)
                emit_superchunk(K)
            drain(g1)
            # interleave canny T2 with superchunks 3-6
            for K in range(3, 7):
                drain(g2, 3)
                emit_superchunk(K)
            drain(g2)
            emit_superchunk(7)

    nc.compile()
    return nc


_NC_CACHE = None


def _host_mats():
    idx = np.arange(128)
    kk, pp = np.meshgrid(idx, idx, indexing="ij")   # [k, p]
    tri121 = np.where(kk == pp, 2.0, 0.0) + np.where(np.abs(kk - pp) == 1, 1.0, 0.0)
    trim101 = np.where(kk == pp + 1, 1.0, 0.0) - np.where(kk == pp - 1, 1.0, 0.0)
    shup = np.where(kk == pp + 1, 1.0, 0.0)
    shdn = np.where(kk == pp - 1, 1.0, 0.0)
    tri111 = np.where(np.abs(kk - pp) <= 1, 1.0, 0.0)
    m = np.stack([tri121, trim101, shup, shdn, tri111]).astype(np.float16)
    return np.ascontiguousarray(m.transpose(1, 0, 2).reshape(128, 5 * 128))


def _prep_in_maps(x, Wc, b):
    x = np.ascontiguousarray(np.asarray(x, dtype=np.float32))
    Wc = np.asarray(Wc, dtype=np.float32)
    b = np.asarray(b, dtype=np.float32)
    # rhs partition order: p = g*3 + c for x channels, p = 6 + g for the edge
    wt8 = np.zeros((8, 128), np.float32)
    for g in range(2):
        wt8[g * 3:g * 3 + 3, g * 64:g * 64 + 64] = Wc[:, 0:3].T
        wt8[6 + g, g * 64:g * 64 + 64] = Wc[:, 3]
    bias128 = np.ascontiguousarray(np.concatenate([b, b]).astype(np.float32)[:, None])
    mats = _host_mats()
    in_maps = []
    for c in range(8):
        img, half = c // 2, c % 2
        S = half * 256
        rows = np.arange(S - 9, S + 265)
        rr = np.abs(rows)
        rr = np.where(rr > 511, 1022 - rr, rr)
        xs = np.ascontiguousarray(x[img][:, rr, :])
        # xb_dev[K, g*3+c, jj*2048+hh*512+w] = x[c, S + 32K+8jj+4g+hh, w]
        xh = x[img][:, S:S + 256, :].astype(np.float16)           # [3, 256, 512]
        xb = np.ascontiguousarray(
            xh.reshape(3, 16, 2, 2, 4, W).transpose(1, 3, 0, 2, 4, 5).reshape(16, 6, 4096))
        mask = ((rows >= 0) & (rows <= 511)).astype(np.float32)
        m3 = np.ascontiguousarray(np.stack([mask[q:q + 128] for q in T_Q]))
        in_maps.append({"xs": xs, "xb": xb, "wt": wt8, "bias": bias128,
                        "mask": m3, "mats": mats})
    return in_maps


def kernel(x, Wc, b):
    global _NC_CACHE, LAST_RESULT
    if _NC_CACHE is None:
        _NC_CACHE = build_nc()
    in_maps = _prep_in_maps(x, Wc, b)
    res = run_bass_kernel_spmd(_NC_CACHE, in_maps, core_ids=list(range(8)))
    LAST_RESULT = res
    out = np.empty((B, 64, H, W), np.float32)
    for c in range(8):
        img, half = c // 2, c % 2
        o = res.results[c]["out"]                      # [8, 128, 8192]
        # partition = g*64+o ; free = jj*2048 + hh*512 + w ; h = 32K+8jj+4g+hh
        o = o.reshape(8, 2, 64, 4, 4, W).transpose(2, 0, 3, 1, 4, 5).reshape(64, 256, W)
        out[img, :, half * 256:(half + 1) * 256, :] = o
    return out


if __name__ == "__main__":
    d = np.load('/tmp/ref_inputs.npz')
    out = kernel(d['x'], d['Wc'], d['b'])
    ref = np.load('/tmp/ref_out.npy')
    err = np.linalg.norm(out - ref) / np.linalg.norm(ref)
    print("rel l2 err:", err, "max abs:", np.abs(out - ref).max())


# revision 53
# speedup vs baseline: 1.1061x; 1.1061x over previous
"""Trainium2 Bass kernel for the Canny-edge + 1x1-conv module.

Sharding: 8 cores = 4 images x 2 row-halves. Each core computes Canny on its
half (3 independent 128-row tiles with halos, K=5 hysteresis iterations) and
streams the fused concat+1x1conv+bias+relu output (32 MB/core) back to HBM.

All vertical (partition-axis) +-1 shifts go through TensorEngine matmuls with
constant banded matrices (compute-engine APs must start at partition 0).
Binary dilation = (3x3 box-sum >= 1), where the vertical part of the box-sum
is a tridiagonal matmul.

Self-contained: hardcodes all shapes; callable as kernel(x=..., Wc=..., b=...).
"""
import numpy as np

import concourse.bass as bass
import concourse.bacc as bacc
import concourse.mybir as mybir
import concourse.tile as tile
from concourse.bass_utils import run_bass_kernel_spmd

F32 = mybir.dt.float32
F16 = mybir.dt.float16
OP = mybir.AluOpType
ACT = mybir.ActivationFunctionType

B, C, H, W = 4, 3, 512, 512
WP = W + 2            # column-padded width
HS = 274              # shard rows: image rows [S-9, S+265)
K_HYST = 4
T_Q = [0, 112, 146]   # canny tile start rows within the shard
N_CHUNK = 32          # output chunks of 8 rows each
MAGIC = 8388608.0     # 2^23: f32 round-to-int trick
T1 = 0.4142135623730951   # tan(22.5 deg)
T2 = 2.414213562373095    # tan(67.5 deg)

LAST_RESULT = None    # BassKernelResults of the most recent run (for test.py)


def _chunk_map(k):
    """output chunk k (rows 8k..8k+8) -> (canny tile idx, partition start)"""
    if k <= 13:
        return 0, 8 * k + 9
    if k <= 27:
        return 1, 8 * k - 103
    return 2, 8 * k - 137


def _canny_gen(nc, pools, xs_param, mask_sb, mats, t, edge):
    """Generator emitting the Canny ops for shard rows [T_Q[t], T_Q[t]+128);
    yields between stages so the driver can interleave tiles / conv chunks.

    Ops are emitted per column-segment (L/R halves) so the serial chain
    pipelines across engines (DVE on one half while PE works the other).
    Padded-coordinate segments: [(1,258), (258,513)]; unpadded tiles use
    [a-1, b-1).
    """
    scr = pools["scratch"]
    cps = pools["cpsum"]
    xtLR = pools["xt"][t]          # [xtL [128,3*257], xtR [128,3*255]]
    SEG = [(1, 258), (258, 513)]

    # ---- gray = trunc(0.2989 x0 + 0.587 x1 + 0.114 x2)  (f32, exact) ----
    gray = scr.tile([128, W], F32, tag="gray")
    ground = scr.tile([128, W], F32, tag="ground")
    cmp = scr.tile([128, W], F16, tag="cmp")
    g = scr.tile([128, WP], F16, tag="g")
    for si, (a, b) in enumerate(SEG):
        u = slice(a - 1, b - 1)
        xts = xtLR[si]
        n = b - a
        nc.vector.tensor_scalar_mul(gray[:, u], xts[:, 0:n], 0.2989)
        nc.vector.scalar_tensor_tensor(gray[:, u], xts[:, n:2 * n], 0.587, gray[:, u], OP.mult, OP.add)
        nc.vector.scalar_tensor_tensor(gray[:, u], xts[:, 2 * n:3 * n], 0.114, gray[:, u], OP.mult, OP.add)
        nc.vector.tensor_scalar(ground[:, u], gray[:, u], MAGIC, MAGIC, OP.add, OP.subtract)
        nc.vector.tensor_tensor(cmp[:, u], ground[:, u], gray[:, u], OP.is_gt)
        nc.vector.tensor_tensor(g[:, a:b], ground[:, u], cmp[:, u], OP.subtract)
    nc.vector.tensor_copy(g[:, 0:1], g[:, 2:3])        # reflect cols
    nc.vector.tensor_copy(g[:, 513:514], g[:, 511:512])
    yield

    # ---- sobel: horizontal parts on DVE, vertical 3-taps via matmul ----
    dcol = scr.tile([128, W], F16, tag="dcol")
    hsm = scr.tile([128, W], F16, tag="hsm")
    gx = scr.tile([128, WP], F16, tag="gx")
    gy = scr.tile([128, WP], F16, tag="gy")
    ax = scr.tile([128, WP], F16, tag="ax")
    ay = scr.tile([128, WP], F16, tag="ay")
    pr = scr.tile([128, WP], F16, tag="pr")
    for (a, b) in SEG:
        u = slice(a - 1, b - 1)
        nc.vector.tensor_sub(dcol[:, u], g[:, a + 1:b + 1], g[:, a - 1:b - 1])
        nc.vector.scalar_tensor_tensor(hsm[:, u], g[:, a:b], 2.0, g[:, a - 1:b - 1], OP.mult, OP.add)
        nc.vector.tensor_add(hsm[:, u], hsm[:, u], g[:, a + 1:b + 1])
    yield
    for (a, b) in SEG:
        u = slice(a - 1, b - 1)
        n = b - a
        ps_gx = cps.tile([128, n], F32, tag="cps", padded_shape=[128, 257])
        nc.tensor.matmul(ps_gx[:, :], mats["tri121"][:, :], dcol[:, u], start=True, stop=True)
        ps_gy = cps.tile([128, n], F32, tag="cps", padded_shape=[128, 257])
        nc.tensor.matmul(ps_gy[:, :], mats["trim101"][:, :], hsm[:, u], start=True, stop=True)
        nc.vector.tensor_copy(gx[:, a:b], ps_gx[:, :])
        nc.scalar.activation(ax[:, a:b], ps_gx[:, :], ACT.Abs)
        nc.vector.tensor_copy(gy[:, a:b], ps_gy[:, :])
        nc.scalar.activation(ay[:, a:b], ps_gy[:, :], ACT.Abs)
        # scale one factor by 2^-6 (exact) so |pr| <= 64516 stays finite in fp16
        nc.vector.scalar_tensor_tensor(pr[:, a:b], gx[:, a:b], 0.015625, gy[:, a:b],
                                       OP.mult, OP.mult)
    yield

    # ---- mag (+ boundary mask) and shifted copies via shift matmuls ----
    mag = scr.tile([128, WP], F16, tag="mag")
    magu = scr.tile([128, WP], F16, tag="magu")
    magd = scr.tile([128, WP], F16, tag="magd")
    U8 = mybir.dt.uint8
    c0 = scr.tile([128, WP], U8, tag="c0")
    c2 = scr.tile([128, WP], U8, tag="c2")
    c45 = scr.tile([128, WP], U8, tag="c45")
    nc.vector.memset(mag[:, 0:1], 0.0)
    nc.vector.memset(mag[:, 513:514], 0.0)
    nc.vector.memset(magu[:, 0:1], 0.0)
    nc.vector.memset(magu[:, 513:514], 0.0)
    nc.vector.memset(magd[:, 0:1], 0.0)
    nc.vector.memset(magd[:, 513:514], 0.0)
    for (a, b) in SEG:
        nc.vector.tensor_add(mag[:, a:b], ax[:, a:b], ay[:, a:b])
        nc.vector.tensor_scalar(mag[:, a:b], mag[:, a:b], mask_sb[:, t:t + 1], None, OP.mult)
    yield
    for (a, b) in SEG:
        n = b - a
        ps_mu = cps.tile([128, n], F32, tag="cps", padded_shape=[128, 257])
        nc.tensor.matmul(ps_mu[:, :], mats["shup"][:, :], mag[:, a:b], start=True, stop=True)
        ps_md = cps.tile([128, n], F32, tag="cps", padded_shape=[128, 257])
        nc.tensor.matmul(ps_md[:, :], mats["shdn"][:, :], mag[:, a:b], start=True, stop=True)
        nc.vector.tensor_copy(magu[:, a:b], ps_mu[:, :])
        nc.vector.tensor_copy(magd[:, a:b], ps_md[:, :])
        nc.vector.scalar_tensor_tensor(c0[:, a:b], ax[:, a:b], T1, ay[:, a:b], OP.mult, OP.is_gt)
        nc.vector.scalar_tensor_tensor(c2[:, a:b], ax[:, a:b], T2, ay[:, a:b], OP.mult, OP.is_lt)
        nc.vector.tensor_scalar(c45[:, a:b], pr[:, a:b], 0.0, None, OP.is_gt)
    yield

    # ---- NMS neighbors via predicated copies (precedence: c45 < c2 < c0) ----
    n1 = scr.tile([128, WP], F16, tag="n1")
    n2 = scr.tile([128, WP], F16, tag="n2")
    q = scr.tile([128, WP], F16, tag="q")
    nms = scr.tile([128, WP], F16, tag="nms")
    strong = scr.tile([128, WP], F16, tag="strong")
    weak = scr.tile([128, WP], F16, tag="weak")
    for (a, b) in SEG:
        def sh(dy, dx):
            m = {-1: magd, 0: mag, 1: magu}[dy]
            return m[:, a + dx:b + dx]
        nc.vector.tensor_copy(n1[:, a:b], sh(-1, -1))
        nc.vector.tensor_copy(n2[:, a:b], sh(1, 1))
        for (c, a1, a2) in ((c45, (-1, 1), (1, -1)), (c2, (1, 0), (-1, 0)), (c0, (0, 1), (0, -1))):
            nc.vector.copy_predicated(n1[:, a:b], c[:, a:b], sh(*a1))
            nc.vector.copy_predicated(n2[:, a:b], c[:, a:b], sh(*a2))
        yield
        nc.vector.tensor_max(q[:, a:b], n1[:, a:b], n2[:, a:b])
        nc.vector.tensor_tensor(q[:, a:b], mag[:, a:b], q[:, a:b], OP.is_ge)
        nc.vector.tensor_mul(nms[:, a:b], mag[:, a:b], q[:, a:b])
        nc.vector.tensor_scalar(strong[:, a:b], nms[:, a:b], 150.0, None, OP.is_gt)
        nc.vector.tensor_scalar(weak[:, a:b], nms[:, a:b], 50.0, None, OP.is_gt)
    nc.vector.memset(strong[:, 0:1], 0.0)
    nc.vector.memset(strong[:, 513:514], 0.0)
    yield

    # ---- hysteresis: s' = weak AND (3x3 box-sum of s >= 1), K iterations.
    # Box-sum via 3 accumulating matmuls over column-shifted views (PE-only).
    sA = scr.tile([128, WP], F16, tag="sA")
    sB = scr.tile([128, WP], F16, tag="sB")
    for sbuf_t in (sA, sB):
        nc.vector.memset(sbuf_t[:, 0:1], 0.0)
        nc.vector.memset(sbuf_t[:, 513:514], 0.0)
    cur = strong
    for it in range(K_HYST):
        nxt = sA if (it % 2 == 0) else sB
        for (a, b) in SEG:
            n = b - a
            ps_h = cps.tile([128, n], F32, tag="cps", padded_shape=[128, 257])
            nc.tensor.matmul(ps_h[:, :], mats["tri111"][:, :], cur[:, a - 1:b - 1], start=True, stop=False)
            nc.tensor.matmul(ps_h[:, :], mats["tri111"][:, :], cur[:, a:b], start=False, stop=False)
            nc.tensor.matmul(ps_h[:, :], mats["tri111"][:, :], cur[:, a + 1:b + 1], start=False, stop=True)
            nc.vector.scalar_tensor_tensor(nxt[:, a:b], ps_h[:, :], 0.5, weak[:, a:b],
                                           OP.is_ge, OP.mult)
        cur = nxt
        yield

    for (a, b) in SEG:
        nc.vector.tensor_scalar(edge[:, a - 1:b - 1], cur[:, a:b], 255.0, None, OP.mult)


def build_nc():
    nc = bacc.Bacc("TRN2", target_bir_lowering=False)
    xs_param = nc.declare_dram_parameter("xs", [3, HS, W], F32, isOutput=False)
    xb_param = nc.declare_dram_parameter("xb", [8, 6, 8192], F16, isOutput=False)
    wt_param = nc.declare_dram_parameter("wt", [8, 128], F32, isOutput=False)
    bias_param = nc.declare_dram_parameter("bias", [128, 1], F32, isOutput=False)
    mask_param = nc.declare_dram_parameter("mask", [3, 128], F32, isOutput=False)
    mats_param = nc.declare_dram_parameter("mats", [128, 5 * 128], F16, isOutput=False)
    out_param = nc.declare_dram_parameter("out", [8, 128, 8192], F32, isOutput=True)

    MAT_NAMES = ["tri121", "trim101", "shup", "shdn", "tri111"]

    with tile.TileContext(nc) as tc:
        import contextlib
        with contextlib.ExitStack() as ctx:
            const = ctx.enter_context(tc.tile_pool(name="const", bufs=1))
            scratch = ctx.enter_context(tc.tile_pool(name="scratch", bufs=2))
            epool = ctx.enter_context(tc.tile_pool(name="edges", bufs=1))
            rhs_pool = ctx.enter_context(tc.tile_pool(name="rhs", bufs=2))
            stage_pool = ctx.enter_context(tc.tile_pool(name="stage", bufs=4))
            psum_pool = ctx.enter_context(tc.tile_pool(name="psum", bufs=2, space="PSUM"))
            cpsum_pool = ctx.enter_context(tc.tile_pool(name="cpsum", bufs=3, space="PSUM"))
            pools = {"scratch": scratch, "cpsum": cpsum_pool}

            wt_sb = const.tile([8, 128], F32, tag="wt")
            lhsT = const.tile([8, 128], F16, tag="lhsT")
            bias_sb = const.tile([128, 1], F32, tag="bias")
            mask_sb = const.tile([128, 3], F32, tag="mask")
            mats_sb = const.tile([128, 5 * 128], F16, tag="mats")
            # canny x tiles first in the sync FIFO (they gate the critical path);
            # split L/R so the first gray ops start as soon as the L half lands
            xts = []
            for t in range(3):
                pair = []
                for si, (ca, cb) in enumerate(((0, 257), (257, 512))):
                    n = cb - ca
                    xt = const.tile([128, 3 * n], F32, tag=f"xt{t}_{si}", name=f"xt{t}_{si}")
                    nc.sync.dma_start(
                        xt[:, :].rearrange("p (c w) -> p c w", c=3),
                        xs_param[:, T_Q[t]:T_Q[t] + 128, ca:cb].rearrange("c h w -> h c w"))
                    pair.append(xt)
                xts.append(pair)
            pools["xt"] = xts
            nc.scalar.dma_start(mats_sb[:, :], mats_param[:, :])
            nc.scalar.dma_start(wt_sb[:, :], wt_param[:, :])
            nc.scalar.dma_start(bias_sb[:, :], bias_param[:, :])
            nc.scalar.dma_start(mask_sb[:, :], mask_param.rearrange("t p -> p t"))
            nc.vector.tensor_copy(lhsT[:, :], wt_sb[:, :])
            mats = {nm: mats_sb[:, 128 * i:128 * (i + 1)] for i, nm in enumerate(MAT_NAMES)}

            edges = [epool.tile([128, W], F16, tag=f"edge{t}", name=f"edge{t}")
                     for t in range(3)]

            # relu fills handled by DVE per superchunk (DVE is canny-busy early)
            DVE_FILLS = {0: (), 1: (), 2: (), 3: (3,), 4: (2, 5), 5: (2, 5), 6: (1, 4, 7), 7: (0, 2, 4, 6)}

            def emit_superchunk(K):
                rhs = rhs_pool.tile([8, 8192], F16, tag="rhs")
                nc.sync.dma_start(rhs[0:6, :], xb_param[K])
                for jj in range(4):
                    k = 4 * K + jj
                    t, p0 = _chunk_map(k)
                    for gg in range(2):
                        nc.sync.dma_start(
                            rhs[6 + gg:7 + gg, 2048 * jj:2048 * (jj + 1)]
                            .rearrange("one (h w) -> one h w", h=4),
                            edges[t][p0 + 4 * gg:p0 + 4 * gg + 4, :],
                        )
                for half in range(2):
                    stage = stage_pool.tile([128, 4096], F32, tag="stage")
                    for jj4 in range(4):
                        jj = half * 4 + jj4
                        psum = psum_pool.tile([128, 1024], F32, tag="psum")
                        for j in range(2):
                            nc.tensor.matmul(psum[:, 512 * j:512 * (j + 1)], lhsT[:, :],
                                             rhs[:, 1024 * jj + 512 * j:1024 * jj + 512 * (j + 1)],
                                             start=True, stop=True)
                        o0 = 1024 * jj4
                        if jj in DVE_FILLS[K]:
                            nc.vector.tensor_scalar(stage[:, o0:o0 + 1024], psum[:, :],
                                                    bias_sb[:, :], 0.0, OP.add, OP.max)
                        else:
                            nc.scalar.activation(stage[:, o0:o0 + 1024], psum[:, :],
                                                 ACT.Relu, bias=bias_sb[:, :])
                    eng = nc.scalar if (2 * K + half) % 2 == 0 else nc.gpsimd
                    eng.dma_start(out_param[K, :, 4096 * half:4096 * (half + 1)], stage[:, :])

            def drain(gen, n=10**9):
                for _ in range(n):
                    if next(gen, "done") == "done":
                        return True
                return False

            g0 = _canny_gen(nc, pools, xs_param, mask_sb, mats, 0, edges[0])
            g1 = _canny_gen(nc, pools, xs_param, mask_sb, mats, 1, edges[1])
            g2 = _canny_gen(nc, pools, xs_param, mask_sb, mats, 2, edges[2])
            drain(g0)
            # interleave canny T1 with superchunks 0-2 (T0-backed)
            for K in range(0, 3):
                drain(g1, 4)
                emit_superchunk(K)
            drain(g1)
            # interleave canny T2 with superchunks 3-6
            for K in range(3, 7):
                drain(g2, 3)
                emit_superchunk(K)
            drain(g2)
            emit_superchunk(7)

    nc.compile()
    return nc


_NC_CACHE = None


def _host_mats():
    idx = np.arange(128)
    kk, pp = np.meshgrid(idx, idx, indexing="ij")   # [k, p]
    tri121 = np.where(kk == pp, 2.0, 0.0) + np.where(np.abs(kk - pp) == 1, 1.0, 0.0)
    trim101 = np.where(kk == pp + 1, 1.0, 0.0) - np.where(kk == pp - 1, 1.0, 0.0)
    shup = np.where(kk == pp + 1, 1.0, 0.0)
    shdn = np.where(kk == pp - 1, 1.0, 0.0)
    tri111 = np.where(np.abs(kk - pp) <= 1, 1.0, 0.0)
    m = np.stack([tri121, trim101, shup, shdn, tri111]).astype(np.float16)
    return np.ascontiguousarray(m.transpose(1, 0, 2).reshape(128, 5 * 128))


def _prep_in_maps(x, Wc, b):
    x = np.ascontiguousarray(np.asarray(x, dtype=np.float32))
    Wc = np.asarray(Wc, dtype=np.float32)
    b = np.asarray(b, dtype=np.float32)
    # rhs partition order: p = g*3 + c for x channels, p = 6 + g for the edge
    wt8 = np.zeros((8, 128), np.float32)
    for g in range(2):
        wt8[g * 3:g * 3 + 3, g * 64:g * 64 + 64] = Wc[:, 0:3].T
        wt8[6 + g, g * 64:g * 64 + 64] = Wc[:, 3]
    bias128 = np.ascontiguousarray(np.concatenate([b, b]).astype(np.float32)[:, None])
    mats = _host_mats()
    in_maps = []
    for c in range(8):
        img, half = c // 2, c % 2
        S = half * 256
        rows = np.arange(S - 9, S + 265)
        rr = np.abs(rows)
        rr = np.where(rr > 511, 1022 - rr, rr)
        xs = np.ascontiguousarray(x[img][:, rr, :])
        # xb_dev[K, g*3+c, jj*2048+hh*512+w] = x[c, S + 32K+8jj+4g+hh, w]
        xh = x[img][:, S:S + 256, :].astype(np.float16)           # [3, 256, 512]
        xb = np.ascontiguousarray(
            xh.reshape(3, 8, 4, 2, 4, W).transpose(1, 3, 0, 2, 4, 5).reshape(8, 6, 8192))
        mask = ((rows >= 0) & (rows <= 511)).astype(np.float32)
        m3 = np.ascontiguousarray(np.stack([mask[q:q + 128] for q in T_Q]))
        in_maps.append({"xs": xs, "xb": xb, "wt": wt8, "bias": bias128,
                        "mask": m3, "mats": mats})
    return in_maps


def kernel(x, Wc, b):
    global _NC_CACHE, LAST_RESULT
    if _NC_CACHE is None:
        _NC_CACHE = build_nc()
    in_maps = _prep_in_maps(x, Wc, b)
    res = run_bass_kernel_spmd(_NC_CACHE, in_maps, core_ids=list(range(8)))
    LAST_RESULT = res
    out = np.empty((B, 64, H, W), np.float32)
    for c in range(8):
        img, half = c // 2, c % 2
        o = res.results[c]["out"]                      # [8, 128, 8192]
        # partition = g*64+o ; free = jj*2048 + hh*512 + w ; h = 32K+8jj+4g+hh
        o = o.reshape(8, 2, 64, 4, 4, W).transpose(2, 0, 3, 1, 4, 5).reshape(64, 256, W)
        out[img, :, half * 256:(half + 1) * 256, :] = o
    return out


if __name__ == "__main__":
    d = np.load('/tmp/ref_inputs.npz')
    out = kernel(d['x'], d['Wc'], d['b'])
    ref = np.load('/tmp/ref_out.npy')
    err = np.linalg.norm(out - ref) / np.linalg.norm(ref)
    print("rel l2 err:", err, "max abs:", np.abs(out - ref).max())


# revision 54
# speedup vs baseline: 1.1426x; 1.0330x over previous
"""Trainium2 Bass kernel for the Canny-edge + 1x1-conv module.

Sharding: 8 cores = 4 images x 2 row-halves. Each core computes Canny on its
half (3 independent 128-row tiles with halos, K=5 hysteresis iterations) and
streams the fused concat+1x1conv+bias+relu output (32 MB/core) back to HBM.

All vertical (partition-axis) +-1 shifts go through TensorEngine matmuls with
constant banded matrices (compute-engine APs must start at partition 0).
Binary dilation = (3x3 box-sum >= 1), where the vertical part of the box-sum
is a tridiagonal matmul.

Self-contained: hardcodes all shapes; callable as kernel(x=..., Wc=..., b=...).
"""
import numpy as np

import concourse.bass as bass
import concourse.bacc as bacc
import concourse.mybir as mybir
import concourse.tile as tile
from concourse.bass_utils import run_bass_kernel_spmd

F32 = mybir.dt.float32
F16 = mybir.dt.float16
OP = mybir.AluOpType
ACT = mybir.ActivationFunctionType

B, C, H, W = 4, 3, 512, 512
WP = W + 2            # column-padded width
HS = 274              # shard rows: image rows [S-9, S+265)
K_HYST = 4
T_Q = [0, 112, 146]   # canny tile start rows within the shard
N_CHUNK = 32          # output chunks of 8 rows each
MAGIC = 8388608.0     # 2^23: f32 round-to-int trick
T1 = 0.4142135623730951   # tan(22.5 deg)
T2 = 2.414213562373095    # tan(67.5 deg)

LAST_RESULT = None    # BassKernelResults of the most recent run (for test.py)


def _chunk_map(k):
    """output chunk k (rows 8k..8k+8) -> (canny tile idx, partition start)"""
    if k <= 13:
        return 0, 8 * k + 9
    if k <= 27:
        return 1, 8 * k - 103
    return 2, 8 * k - 137


def _canny_gen(nc, pools, xs_param, mask_sb, mats, t, edge):
    copy_eng = nc.scalar if t == 0 else nc.vector
    def psum_copy(dst, src):
        if t == 0:
            nc.scalar.copy(dst, src)
        else:
            nc.vector.tensor_copy(dst, src)
    """Generator emitting the Canny ops for shard rows [T_Q[t], T_Q[t]+128);
    yields between stages so the driver can interleave tiles / conv chunks.

    Ops are emitted per column-segment (L/R halves) so the serial chain
    pipelines across engines (DVE on one half while PE works the other).
    Padded-coordinate segments: [(1,258), (258,513)]; unpadded tiles use
    [a-1, b-1).
    """
    scr = pools["scratch"]
    cps = pools["cpsum"]
    xtLR = pools["xt"][t]          # [xtL [128,3*257], xtR [128,3*255]]
    SEG = [(1, 258), (258, 513)]

    # ---- gray = trunc(0.2989 x0 + 0.587 x1 + 0.114 x2)  (f32, exact) ----
    gray = scr.tile([128, W], F32, tag="gray")
    ground = scr.tile([128, W], F32, tag="ground")
    cmp = scr.tile([128, W], F16, tag="cmp")
    g = scr.tile([128, WP], F16, tag="g")
    for si, (a, b) in enumerate(SEG):
        u = slice(a - 1, b - 1)
        xts = xtLR[si]
        n = b - a
        nc.vector.tensor_scalar_mul(gray[:, u], xts[:, 0:n], 0.2989)
        nc.vector.scalar_tensor_tensor(gray[:, u], xts[:, n:2 * n], 0.587, gray[:, u], OP.mult, OP.add)
        nc.vector.scalar_tensor_tensor(gray[:, u], xts[:, 2 * n:3 * n], 0.114, gray[:, u], OP.mult, OP.add)
        nc.vector.tensor_scalar(ground[:, u], gray[:, u], MAGIC, MAGIC, OP.add, OP.subtract)
        nc.vector.tensor_tensor(cmp[:, u], ground[:, u], gray[:, u], OP.is_gt)
        nc.vector.tensor_tensor(g[:, a:b], ground[:, u], cmp[:, u], OP.subtract)
    nc.vector.tensor_copy(g[:, 0:1], g[:, 2:3])        # reflect cols
    nc.vector.tensor_copy(g[:, 513:514], g[:, 511:512])
    yield

    # ---- sobel: horizontal parts on DVE, vertical 3-taps via matmul ----
    dcol = scr.tile([128, W], F16, tag="dcol")
    hsm = scr.tile([128, W], F16, tag="hsm")
    gx = scr.tile([128, WP], F16, tag="gx")
    gy = scr.tile([128, WP], F16, tag="gy")
    ax = scr.tile([128, WP], F16, tag="ax")
    ay = scr.tile([128, WP], F16, tag="ay")
    pr = scr.tile([128, WP], F16, tag="pr")
    for (a, b) in SEG:
        u = slice(a - 1, b - 1)
        nc.vector.tensor_sub(dcol[:, u], g[:, a + 1:b + 1], g[:, a - 1:b - 1])
        nc.vector.scalar_tensor_tensor(hsm[:, u], g[:, a:b], 2.0, g[:, a - 1:b - 1], OP.mult, OP.add)
        nc.vector.tensor_add(hsm[:, u], hsm[:, u], g[:, a + 1:b + 1])
    yield
    for (a, b) in SEG:
        u = slice(a - 1, b - 1)
        n = b - a
        ps_gx = cps.tile([128, n], F32, tag="cps", padded_shape=[128, 257])
        nc.tensor.matmul(ps_gx[:, :], mats["tri121"][:, :], dcol[:, u], start=True, stop=True)
        ps_gy = cps.tile([128, n], F32, tag="cps", padded_shape=[128, 257])
        nc.tensor.matmul(ps_gy[:, :], mats["trim101"][:, :], hsm[:, u], start=True, stop=True)
        psum_copy(gx[:, a:b], ps_gx[:, :])
        nc.scalar.activation(ax[:, a:b], ps_gx[:, :], ACT.Abs)
        psum_copy(gy[:, a:b], ps_gy[:, :])
        nc.scalar.activation(ay[:, a:b], ps_gy[:, :], ACT.Abs)
        # scale one factor by 2^-6 (exact) so |pr| <= 64516 stays finite in fp16
        nc.vector.scalar_tensor_tensor(pr[:, a:b], gx[:, a:b], 0.015625, gy[:, a:b],
                                       OP.mult, OP.mult)
    yield

    # ---- mag (+ boundary mask) and shifted copies via shift matmuls ----
    mag = scr.tile([128, WP], F16, tag="mag")
    magu = scr.tile([128, WP], F16, tag="magu")
    magd = scr.tile([128, WP], F16, tag="magd")
    U8 = mybir.dt.uint8
    c0 = scr.tile([128, WP], U8, tag="c0")
    c2 = scr.tile([128, WP], U8, tag="c2")
    c45 = scr.tile([128, WP], U8, tag="c45")
    nc.vector.memset(mag[:, 0:1], 0.0)
    nc.vector.memset(mag[:, 513:514], 0.0)
    nc.vector.memset(magu[:, 0:1], 0.0)
    nc.vector.memset(magu[:, 513:514], 0.0)
    nc.vector.memset(magd[:, 0:1], 0.0)
    nc.vector.memset(magd[:, 513:514], 0.0)
    for (a, b) in SEG:
        nc.vector.tensor_add(mag[:, a:b], ax[:, a:b], ay[:, a:b])
        nc.vector.tensor_scalar(mag[:, a:b], mag[:, a:b], mask_sb[:, t:t + 1], None, OP.mult)
    yield
    for (a, b) in SEG:
        n = b - a
        ps_mu = cps.tile([128, n], F32, tag="cps", padded_shape=[128, 257])
        nc.tensor.matmul(ps_mu[:, :], mats["shup"][:, :], mag[:, a:b], start=True, stop=True)
        ps_md = cps.tile([128, n], F32, tag="cps", padded_shape=[128, 257])
        nc.tensor.matmul(ps_md[:, :], mats["shdn"][:, :], mag[:, a:b], start=True, stop=True)
        psum_copy(magu[:, a:b], ps_mu[:, :])
        psum_copy(magd[:, a:b], ps_md[:, :])
        nc.vector.scalar_tensor_tensor(c0[:, a:b], ax[:, a:b], T1, ay[:, a:b], OP.mult, OP.is_gt)
        nc.vector.scalar_tensor_tensor(c2[:, a:b], ax[:, a:b], T2, ay[:, a:b], OP.mult, OP.is_lt)
        nc.vector.tensor_scalar(c45[:, a:b], pr[:, a:b], 0.0, None, OP.is_gt)
    yield

    # ---- NMS neighbors via predicated copies (precedence: c45 < c2 < c0) ----
    n1 = scr.tile([128, WP], F16, tag="n1")
    n2 = scr.tile([128, WP], F16, tag="n2")
    q = scr.tile([128, WP], F16, tag="q")
    nms = scr.tile([128, WP], F16, tag="nms")
    strong = scr.tile([128, WP], F16, tag="strong")
    weak = scr.tile([128, WP], F16, tag="weak")
    for (a, b) in SEG:
        def sh(dy, dx):
            m = {-1: magd, 0: mag, 1: magu}[dy]
            return m[:, a + dx:b + dx]
        nc.vector.tensor_copy(n1[:, a:b], sh(-1, -1))
        nc.vector.tensor_copy(n2[:, a:b], sh(1, 1))
        for (c, a1, a2) in ((c45, (-1, 1), (1, -1)), (c2, (1, 0), (-1, 0)), (c0, (0, 1), (0, -1))):
            nc.vector.copy_predicated(n1[:, a:b], c[:, a:b], sh(*a1))
            nc.vector.copy_predicated(n2[:, a:b], c[:, a:b], sh(*a2))
        yield
        nc.vector.tensor_max(q[:, a:b], n1[:, a:b], n2[:, a:b])
        nc.vector.tensor_tensor(q[:, a:b], mag[:, a:b], q[:, a:b], OP.is_ge)
        nc.vector.tensor_mul(nms[:, a:b], mag[:, a:b], q[:, a:b])
        nc.vector.tensor_scalar(strong[:, a:b], nms[:, a:b], 150.0, None, OP.is_gt)
        nc.vector.tensor_scalar(weak[:, a:b], nms[:, a:b], 50.0, None, OP.is_gt)
    nc.vector.memset(strong[:, 0:1], 0.0)
    nc.vector.memset(strong[:, 513:514], 0.0)
    yield

    # ---- hysteresis: s' = weak AND (3x3 box-sum of s >= 1), K iterations.
    # Box-sum via 3 accumulating matmuls over column-shifted views (PE-only).
    sA = scr.tile([128, WP], F16, tag="sA")
    sB = scr.tile([128, WP], F16, tag="sB")
    for sbuf_t in (sA, sB):
        nc.vector.memset(sbuf_t[:, 0:1], 0.0)
        nc.vector.memset(sbuf_t[:, 513:514], 0.0)
    cur = strong
    for it in range(K_HYST):
        nxt = sA if (it % 2 == 0) else sB
        for (a, b) in SEG:
            n = b - a
            ps_h = cps.tile([128, n], F32, tag="cps", padded_shape=[128, 257])
            nc.tensor.matmul(ps_h[:, :], mats["tri111"][:, :], cur[:, a - 1:b - 1], start=True, stop=False)
            nc.tensor.matmul(ps_h[:, :], mats["tri111"][:, :], cur[:, a:b], start=False, stop=False)
            nc.tensor.matmul(ps_h[:, :], mats["tri111"][:, :], cur[:, a + 1:b + 1], start=False, stop=True)
            nc.vector.scalar_tensor_tensor(nxt[:, a:b], ps_h[:, :], 0.5, weak[:, a:b],
                                           OP.is_ge, OP.mult)
        cur = nxt
        yield

    for (a, b) in SEG:
        nc.vector.tensor_scalar(edge[:, a - 1:b - 1], cur[:, a:b], 255.0, None, OP.mult)


def build_nc():
    nc = bacc.Bacc("TRN2", target_bir_lowering=False)
    xs_param = nc.declare_dram_parameter("xs", [3, HS, W], F32, isOutput=False)
    xb_param = nc.declare_dram_parameter("xb", [8, 6, 8192], F16, isOutput=False)
    wt_param = nc.declare_dram_parameter("wt", [8, 128], F32, isOutput=False)
    bias_param = nc.declare_dram_parameter("bias", [128, 1], F32, isOutput=False)
    mask_param = nc.declare_dram_parameter("mask", [3, 128], F32, isOutput=False)
    mats_param = nc.declare_dram_parameter("mats", [128, 5 * 128], F16, isOutput=False)
    out_param = nc.declare_dram_parameter("out", [8, 128, 8192], F32, isOutput=True)

    MAT_NAMES = ["tri121", "trim101", "shup", "shdn", "tri111"]

    with tile.TileContext(nc) as tc:
        import contextlib
        with contextlib.ExitStack() as ctx:
            const = ctx.enter_context(tc.tile_pool(name="const", bufs=1))
            scratch = ctx.enter_context(tc.tile_pool(name="scratch", bufs=2))
            epool = ctx.enter_context(tc.tile_pool(name="edges", bufs=1))
            rhs_pool = ctx.enter_context(tc.tile_pool(name="rhs", bufs=2))
            stage_pool = ctx.enter_context(tc.tile_pool(name="stage", bufs=4))
            psum_pool = ctx.enter_context(tc.tile_pool(name="psum", bufs=2, space="PSUM"))
            cpsum_pool = ctx.enter_context(tc.tile_pool(name="cpsum", bufs=3, space="PSUM"))
            pools = {"scratch": scratch, "cpsum": cpsum_pool}

            wt_sb = const.tile([8, 128], F32, tag="wt")
            lhsT = const.tile([8, 128], F16, tag="lhsT")
            bias_sb = const.tile([128, 1], F32, tag="bias")
            mask_sb = const.tile([128, 3], F32, tag="mask")
            mats_sb = const.tile([128, 5 * 128], F16, tag="mats")
            # canny x tiles first in the sync FIFO (they gate the critical path);
            # split L/R so the first gray ops start as soon as the L half lands
            xts = []
            for t in range(3):
                pair = []
                for si, (ca, cb) in enumerate(((0, 257), (257, 512))):
                    n = cb - ca
                    xt = const.tile([128, 3 * n], F32, tag=f"xt{t}_{si}", name=f"xt{t}_{si}")
                    nc.sync.dma_start(
                        xt[:, :].rearrange("p (c w) -> p c w", c=3),
                        xs_param[:, T_Q[t]:T_Q[t] + 128, ca:cb].rearrange("c h w -> h c w"))
                    pair.append(xt)
                xts.append(pair)
            pools["xt"] = xts
            nc.scalar.dma_start(mats_sb[:, :], mats_param[:, :])
            nc.scalar.dma_start(wt_sb[:, :], wt_param[:, :])
            nc.scalar.dma_start(bias_sb[:, :], bias_param[:, :])
            nc.scalar.dma_start(mask_sb[:, :], mask_param.rearrange("t p -> p t"))
            nc.vector.tensor_copy(lhsT[:, :], wt_sb[:, :])
            mats = {nm: mats_sb[:, 128 * i:128 * (i + 1)] for i, nm in enumerate(MAT_NAMES)}

            edges = [epool.tile([128, W], F16, tag=f"edge{t}", name=f"edge{t}")
                     for t in range(3)]

            # relu fills handled by DVE per superchunk (DVE is canny-busy early)
            DVE_FILLS = {0: (), 1: (), 2: (), 3: (3,), 4: (2, 5), 5: (2, 5), 6: (1, 4, 7), 7: (0, 2, 4, 6)}

            def emit_superchunk(K):
                rhs = rhs_pool.tile([8, 8192], F16, tag="rhs")
                nc.sync.dma_start(rhs[0:6, :], xb_param[K])
                for jj in range(4):
                    k = 4 * K + jj
                    t, p0 = _chunk_map(k)
                    for gg in range(2):
                        nc.sync.dma_start(
                            rhs[6 + gg:7 + gg, 2048 * jj:2048 * (jj + 1)]
                            .rearrange("one (h w) -> one h w", h=4),
                            edges[t][p0 + 4 * gg:p0 + 4 * gg + 4, :],
                        )
                for half in range(2):
                    stage = stage_pool.tile([128, 4096], F32, tag="stage")
                    for jj4 in range(4):
                        jj = half * 4 + jj4
                        psum = psum_pool.tile([128, 1024], F32, tag="psum")
                        for j in range(2):
                            nc.tensor.matmul(psum[:, 512 * j:512 * (j + 1)], lhsT[:, :],
                                             rhs[:, 1024 * jj + 512 * j:1024 * jj + 512 * (j + 1)],
                                             start=True, stop=True)
                        o0 = 1024 * jj4
                        if jj in DVE_FILLS[K]:
                            nc.vector.tensor_scalar(stage[:, o0:o0 + 1024], psum[:, :],
                                                    bias_sb[:, :], 0.0, OP.add, OP.max)
                        else:
                            nc.scalar.activation(stage[:, o0:o0 + 1024], psum[:, :],
                                                 ACT.Relu, bias=bias_sb[:, :])
                    eng = nc.scalar if (2 * K + half) % 2 == 0 else nc.gpsimd
                    eng.dma_start(out_param[K, :, 4096 * half:4096 * (half + 1)], stage[:, :])

            def drain(gen, n=10**9):
                for _ in range(n):
                    if next(gen, "done") == "done":
                        return True
                return False

            g0 = _canny_gen(nc, pools, xs_param, mask_sb, mats, 0, edges[0])
            g1 = _canny_gen(nc, pools, xs_param, mask_sb, mats, 1, edges[1])
            g2 = _canny_gen(nc, pools, xs_param, mask_sb, mats, 2, edges[2])
            drain(g0)
            # superchunk ops first, canny slices after them in each engine stream
            for K in range(0, 3):
                emit_superchunk(K)
                drain(g1, 4)
            drain(g1)
            for K in range(3, 7):
                emit_superchunk(K)
                drain(g2, 3)
            drain(g2)
            emit_superchunk(7)

    nc.compile()
    return nc


_NC_CACHE = None


def _host_mats():
    idx = np.arange(128)
    kk, pp = np.meshgrid(idx, idx, indexing="ij")   # [k, p]
    tri121 = np.where(kk == pp, 2.0, 0.0) + np.where(np.abs(kk - pp) == 1, 1.0, 0.0)
    trim101 = np.where(kk == pp + 1, 1.0, 0.0) - np.where(kk == pp - 1, 1.0, 0.0)
    shup = np.where(kk == pp + 1, 1.0, 0.0)
    shdn = np.where(kk == pp - 1, 1.0, 0.0)
    tri111 = np.where(np.abs(kk - pp) <= 1, 1.0, 0.0)
    m = np.stack([tri121, trim101, shup, shdn, tri111]).astype(np.float16)
    return np.ascontiguousarray(m.transpose(1, 0, 2).reshape(128, 5 * 128))


def _prep_in_maps(x, Wc, b):
    x = np.ascontiguousarray(np.asarray(x, dtype=np.float32))
    Wc = np.asarray(Wc, dtype=np.float32)
    b = np.asarray(b, dtype=np.float32)
    # rhs partition order: p = g*3 + c for x channels, p = 6 + g for the edge
    wt8 = np.zeros((8, 128), np.float32)
    for g in range(2):
        wt8[g * 3:g * 3 + 3, g * 64:g * 64 + 64] = Wc[:, 0:3].T
        wt8[6 + g, g * 64:g * 64 + 64] = Wc[:, 3]
    bias128 = np.ascontiguousarray(np.concatenate([b, b]).astype(np.float32)[:, None])
    mats = _host_mats()
    in_maps = []
    for c in range(8):
        img, half = c // 2, c % 2
        S = half * 256
        rows = np.arange(S - 9, S + 265)
        rr = np.abs(rows)
        rr = np.where(rr > 511, 1022 - rr, rr)
        xs = np.ascontiguousarray(x[img][:, rr, :])
        # xb_dev[K, g*3+c, jj*2048+hh*512+w] = x[c, S + 32K+8jj+4g+hh, w]
        xh = x[img][:, S:S + 256, :].astype(np.float16)           # [3, 256, 512]
        xb = np.ascontiguousarray(
            xh.reshape(3, 8, 4, 2, 4, W).transpose(1, 3, 0, 2, 4, 5).reshape(8, 6, 8192))
        mask = ((rows >= 0) & (rows <= 511)).astype(np.float32)
        m3 = np.ascontiguousarray(np.stack([mask[q:q + 128] for q in T_Q]))
        in_maps.append({"xs": xs, "xb": xb, "wt": wt8, "bias": bias128,
                        "mask": m3, "mats": mats})
    return in_maps


def kernel(x, Wc, b):
    global _NC_CACHE, LAST_RESULT
    if _NC_CACHE is None:
        _NC_CACHE = build_nc()
    in_maps = _prep_in_maps(x, Wc, b)
    res = run_bass_kernel_spmd(_NC_CACHE, in_maps, core_ids=list(range(8)))
    LAST_RESULT = res
    out = np.empty((B, 64, H, W), np.float32)
    for c in range(8):
        img, half = c // 2, c % 2
        o = res.results[c]["out"]                      # [8, 128, 8192]
        # partition = g*64+o ; free = jj*2048 + hh*512 + w ; h = 32K+8jj+4g+hh
        o = o.reshape(8, 2, 64, 4, 4, W).transpose(2, 0, 3, 1, 4, 5).reshape(64, 256, W)
        out[img, :, half * 256:(half + 1) * 256, :] = o
    return out


if __name__ == "__main__":
    d = np.load('/tmp/ref_inputs.npz')
    out = kernel(d['x'], d['Wc'], d['b'])
    ref = np.load('/tmp/ref_out.npy')
    err = np.linalg.norm(out - ref) / np.linalg.norm(ref)
    print("rel l2 err:", err, "max abs:", np.abs(out - ref).max())
